# revision 20
# baseline (speedup 1.0000x reference)
"""Trainium2 Bass kernel for nn_Attention_57406532878693 (pooling attention).

Math (per (b, h) slice; T=2048, N=128, K2=16):
    x      = hyp[:, b, h*128:(h+1)*128]                    # (T, N)
    m      = x.mean(0)                                     # (N,)
    gx     = tanh(x @ W_w.T + W_b)                         # (T, K2)
    gm     = tanh(Wm_w @ m + Wm_b)                         # (K2,)
    u      = Wh_w[0] * gm                                  # (K2,)
    l      = gx @ u + Wh_b                                 # (T,)
    p      = exp(l)          (no max-sub needed: |l| <= 4.25, tanh-bounded)
    c      = (p @ x) / p.sum()                             # (N,)
    out[b, h*128:(h+1)*128] = c

Sharding: data-parallel over B across 8 cores (4 batches per core).

v3 design (vs the 368us PE-transpose baseline): the old kernel spent its
time on 1288 PE instructions (512 [128,128] PE transposes + 512 small
gate matmuls with per-matmul stationary reload).  This version:

  - casts x to bf16 once (ACT/DVE copy pass) and transposes with the DMA
    X-bar (dma_start(transpose=True)) instead of the PE: natb tiles are
    laid out head-major [t128, (q, c, n)] so ONE xbar DMA per [128,2048]
    tile emits xt tiles [n128, (i=4q+c, t)] where each head's 512
    t-columns are contiguous; 32 xbar DMAs/core replace 512 PE
    transposes (and their PSUM evacuation ALU pass).
  - gate matmuls run stationary-weight-style in bf16: out.T layout
    gxT[k, t] with lhsT = [W_w.T | 0] (M=32, zero-pad kills PSUM
    garbage), rhs = xt 512-column chunks, col-tiled 4 heads concurrent
    via tile_position=(0, 32q).  128 instrs instead of 512.
  - the time-mean path rides the same xt stream: a second accumulating
    matmul per chunk with lhsT = [Wm_w.T/T | 0] sums Wm@x over chunks in
    PSUM; one DVE free-reduce + tanh + mul gives u per head at
    partitions 32q+k, which a constant block-mask turns into the
    block-diagonal U4 [128, 4].
  - logits come out t-major directly: lhsT = tanh(gxT) 128-col chunk,
    rhs = U4 -> l [t128, 4 heads]; exp + accum_out and the
    p_quad/over-read weighted-sum structure are unchanged from the
    baseline, except the wsum runs bf16 with natb as rhs.
"""

import os
import numpy as np

T, B, D = 2048, 32, 1024
H, N, K2 = 8, 128, 16
NCORES = 8
BL = B // NCORES          # 4 batches per core
TC = T // 128             # 16 t-chunks of 128
NQ = 4                    # nat tiles per (batch, head-quad); each holds 512 t
QW = 4 * N                # 512 cols per head-quad

LAST_RESULT = {}          # exec_time_ns etc. for test harness introspection


def _build(nc, tile, mybir, bass, whb_val, repeat=1, loop_n=0):
    f32 = mybir.dt.float32
    f32r = mybir.dt.float32r
    bf16 = mybir.dt.bfloat16
    AF = mybir.ActivationFunctionType

    dbg = os.environ.get("KB_DEBUG", "0") == "1"
    hyp_s = nc.dram_tensor("hyp_s", [T, BL, D], f32, kind="ExternalInput").ap()
    if dbg:
        dbg_l = nc.dram_tensor("dbg_l", [128, 64], f32,
                               kind="ExternalOutput").ap()
        dbg_u = nc.dram_tensor("dbg_u", [128, 4], f32,
                               kind="ExternalOutput").ap()
        dbg_wm = nc.dram_tensor("dbg_wm", [128, 1], f32,
                                kind="ExternalOutput").ap()
        dbg_p = nc.dram_tensor("dbg_p", [128, 144], f32,
                               kind="ExternalOutput").ap()
        dbg_g = nc.dram_tensor("dbg_g", [128, 2048], f32,
                               kind="ExternalOutput").ap()
        dbg_xt = nc.dram_tensor("dbg_xt", [128, 2304], f32,
                                kind="ExternalOutput").ap()
        dbg_nb = nc.dram_tensor("dbg_nb", [128, 2048], mybir.dt.bfloat16,
                                kind="ExternalOutput").ap()
        dbg_xtr = nc.dram_tensor("dbg_xtr", [128, 2304], mybir.dt.bfloat16,
                                 kind="ExternalOutput").ap()
    wgz_d = nc.dram_tensor("wgz", [N, 32], bf16, kind="ExternalInput").ap()
    wmz_d = nc.dram_tensor("wmz", [N, 32], bf16, kind="ExternalInput").ap()
    wbc_d = nc.dram_tensor("wb_col", [128, 1], f32, kind="ExternalInput").ap()
    wmbc_d = nc.dram_tensor("wmb_col", [128, 1], f32, kind="ExternalInput").ap()
    whwm_d = nc.dram_tensor("whw_mask", [128, 4], f32, kind="ExternalInput").ap()
    ones_d = nc.dram_tensor("ones_col", [128, 2], f32, kind="ExternalInput").ap()
    out_s = nc.dram_tensor("out_s", [BL, D], f32, kind="ExternalOutput").ap()

    def r(ap):
        return ap.bitcast(f32r)

    with tile.TileContext(nc) as tc:
        from contextlib import ExitStack

        with ExitStack() as ctx:
            natf_b = int(os.environ.get("KB_NATF", "10"))
            natb_b = int(os.environ.get("KB_NATB", "4"))
            xt_b = int(os.environ.get("KB_XT", "6"))
            g_b = int(os.environ.get("KB_G", "3"))
            psa_b = int(os.environ.get("KB_PSA", "2"))
            psb_b = int(os.environ.get("KB_PSB", "2"))
            psc_b = int(os.environ.get("KB_PSC", "2"))
            psw_b = int(os.environ.get("KB_PSW", "2"))
            cpool = ctx.enter_context(tc.tile_pool(name="consts", bufs=1))
            natf_pool = ctx.enter_context(tc.tile_pool(name="natf", bufs=natf_b))
            natb_pool = ctx.enter_context(tc.tile_pool(name="natb", bufs=natb_b))
            xt_pool = ctx.enter_context(tc.tile_pool(name="xt", bufs=xt_b))
            g_pool = ctx.enter_context(tc.tile_pool(name="g", bufs=g_b))
            sm_pool = ctx.enter_context(tc.tile_pool(name="small", bufs=6))
            out_pool = ctx.enter_context(tc.tile_pool(name="outp", bufs=1))
            psa_pool = ctx.enter_context(
                tc.tile_pool(name="psa", bufs=psa_b, space="PSUM"))
            psb_pool = ctx.enter_context(
                tc.tile_pool(name="psb", bufs=psb_b, space="PSUM"))
            psc_pool = ctx.enter_context(
                tc.tile_pool(name="psc", bufs=psc_b, space="PSUM"))
            psw_pool = ctx.enter_context(
                tc.tile_pool(name="psw", bufs=psw_b, space="PSUM"))

            wgz = cpool.tile([N, 32], bf16, tag="wgz")
            nc.sync.dma_start(wgz[:], wgz_d)
            wmz = cpool.tile([N, 32], bf16, tag="wmz")
            nc.sync.dma_start(wmz[:], wmz_d)
            wbc = cpool.tile([128, 1], f32, tag="wbc")
            nc.sync.dma_start(wbc[:], wbc_d)
            wmbc = cpool.tile([128, 1], f32, tag="wmbc")
            nc.sync.dma_start(wmbc[:], wmbc_d)
            whwm = cpool.tile([128, 4], f32, tag="whwm")
            nc.sync.dma_start(whwm[:], whwm_d)
            ones_c = cpool.tile([128, 2], f32, tag="ones")
            nc.sync.dma_start(r(ones_c[:]), r(ones_d))
            whb_c = cpool.tile([128, 1], f32, tag="whb")
            nc.gpsimd.memset(whb_c[:], float(whb_val))
            # pad cpool to 512B so downstream pool bases (esp. the xbar
            # transpose destinations in xt_pool) are 512B-aligned — the
            # X-bar mangles blocks written to a 32-mod-64 base.
            pad_c = cpool.tile([128, 56], f32, tag="pad512")

            out_sb = out_pool.tile([97, BL * D // 4], f32, tag="out")

            def load_nat(b):
                tiles = []
                for hq in range(2):
                    row = []
                    for j in range(NQ):
                        t0 = j * 4 * 128
                        nt = natf_pool.tile([128, 4 * QW], f32, tag="natf")
                        src = hyp_s[t0:t0 + 4 * 128, b:b + 1,
                                    hq * QW:(hq + 1) * QW].rearrange(
                            "(c p) one d -> p c (one d)", p=128)
                        nc.sync.dma_start(
                            r(nt[:].rearrange("p (c d) -> p c d", c=4)),
                            r(src))
                        row.append(nt)
                    tiles.append(row)
                return tiles

            def downcast(natf, it):
                """natf [128,(c,q,n)] f32 -> natb [128,(q,c,n)] bf16."""
                natb = []
                for hq in range(2):
                    row = []
                    for j in range(NQ):
                        nf = natf[hq][j]
                        nb = natb_pool.tile([128, 4 * QW], bf16, tag="natb")
                        src = nf[:].rearrange(
                            "p (c q n) -> p q c n", q=4, n=N)
                        dst = nb[:].rearrange(
                            "p (q c n) -> p q c n", q=4, n=N)
                        dc = os.environ.get("KB_DC", "mix")
                        use_act = (dc == "act" or
                                   (dc == "mix" and
                                    (hq * NQ + j + it) % 2 == 0))
                        if use_act:
                            nc.scalar.activation(dst, src, AF.Copy)
                        else:
                            nc.vector.tensor_scalar(
                                dst, src, 1.0, 0.0,
                                op0=mybir.AluOpType.mult,
                                op1=mybir.AluOpType.add)
                        row.append(nb)
                    natb.append(row)
                return natb

            def do_batch(b, natf, natb):
                for hq in range(2):
                    # X-bar transpose: one DMA per [128, 2048] tile.
                    # The xbar transposes each 128-col block of the input
                    # separately into the out AP's middle dim; the 132
                    # stride pads blocks apart so .opt() can't collapse
                    # the 3D AP back to 2D (which would overwrite one
                    # [128,128] region 16 times).
                    xts = []
                    for j in range(NQ):
                        xt = xt_pool.tile([128, 16 * 144], bf16, tag="xt")
                        dst = xt[:].rearrange(
                            "n (i t) -> n i t", t=144)[:, :, 0:128]
                        nc.sync.dma_start(dst, natb[hq][j][:],
                                          transpose=True)
                        xts.append(xt)

                    g_sb = g_pool.tile([128, T], bf16, tag="g")
                    psB = psb_pool.tile([128, 512], f32, tag="psb")
                    for j in range(NQ):
                        psA = psa_pool.tile([128, 512], f32, tag="psa")
                        xtv = xts[j][:].rearrange("n (i t) -> n i t", t=144)
                        for q in range(4):
                            rhs = xtv[:, 4 * q:4 * q + 4, 0:128]
                            nc.tensor.matmul(
                                psA[32 * q:32 * q + 32, :], wgz[:], rhs,
                                start=True, stop=True,
                                tile_position=(0, 32 * q),
                                skip_group_check=True)
                        for q in range(4):
                            rhs = xtv[:, 4 * q:4 * q + 4, 0:128]
                            nc.tensor.matmul(
                                psB[32 * q:32 * q + 32, :], wmz[:], rhs,
                                start=(j == 0), stop=(j == NQ - 1),
                                tile_position=(0, 32 * q),
                                skip_group_check=True)
                        nc.scalar.activation(
                            g_sb[:, 512 * j:512 * (j + 1)], psA[:],
                            AF.Tanh, bias=wbc[:])

                    # mean-gate path: u at partitions 32q+k, col-masked to
                    # the block-diagonal U4 [128, 4] (bf16).
                    wmred = sm_pool.tile([128, 1], f32, tag="wmred")
                    nc.vector.tensor_reduce(
                        wmred[:], psB[:],
                        axis=mybir.AxisListType.X, op=mybir.AluOpType.add)
                    tanhc = sm_pool.tile([128, 1], f32, tag="tanhc")
                    nc.scalar.activation(tanhc[:], wmred[:], AF.Tanh,
                                         bias=wmbc[:])
                    u4 = sm_pool.tile([128, 4], bf16, tag="u4")
                    nc.vector.tensor_mul(
                        u4[:], whwm[:], tanhc[:].broadcast_to([128, 4]))

                    # logits t-major: l[t, q] = sum_p g[p, t] * U4[p, q]
                    psC = psc_pool.tile([128, 68], f32, tag="psc")
                    for c in range(TC):
                        nc.tensor.matmul(
                            psC[:, 4 * c:4 * c + 4],
                            g_sb[:, 128 * c:128 * (c + 1)], u4[:],
                            start=True, stop=True, skip_group_check=True)

                    p_quad = sm_pool.tile([128, 144], f32, tag="p_quad")
                    pr_quad = sm_pool.tile([128, 97], f32, tag="pr_quad")
                    nc.gpsimd.memset(p_quad[:], 0.0)
                    nc.gpsimd.memset(pr_quad[:], 1.0)
                    lview = psC[:, 0:64].rearrange("p (c q) -> p q c", q=4)
                    with nc.allow_low_precision(
                            reason="f32r accum is fp32-width"):
                        for q in range(4):
                            nc.scalar.activation(
                                r(p_quad[:, 32 * q:32 * q + TC].unsqueeze(1)),
                                lview[:, q:q + 1, :],
                                AF.Exp, bias=whb_c[:],
                                accum_out=r(pr_quad[:, 32 * q:32 * q + 1]))

                    if dbg and b == 0 and hq == 0:
                        dl = sm_pool.tile([128, 64], f32, tag="dbg")
                        nc.vector.tensor_scalar(
                            dl[:], psC[:, 0:64], 1.0, 0.0,
                            op0=mybir.AluOpType.mult, op1=mybir.AluOpType.add)
                        nc.sync.dma_start(dbg_l, dl[:])
                        du = sm_pool.tile([128, 4], f32, tag="dbgu")
                        nc.vector.tensor_scalar(
                            du[:], u4[:], 1.0, 0.0,
                            op0=mybir.AluOpType.mult, op1=mybir.AluOpType.add)
                        nc.sync.dma_start(dbg_u, du[:])
                        nc.sync.dma_start(dbg_wm, wmred[:])
                        dg = g_pool.tile([128, 2048], f32, tag="dbgg")
                        nc.vector.tensor_scalar(
                            dg[:], g_sb[:], 1.0, 0.0,
                            op0=mybir.AluOpType.mult, op1=mybir.AluOpType.add)
                        nc.sync.dma_start(dbg_g, dg[:])
                        dxt = g_pool.tile([128, 2304], f32, tag="dbgxt")
                        nc.vector.tensor_scalar(
                            dxt[:], xts[0][:], 1.0, 0.0,
                            op0=mybir.AluOpType.mult, op1=mybir.AluOpType.add)
                        nc.sync.dma_start(dbg_xt, dxt[:])
                        nc.sync.dma_start(dbg_nb, natb[0][0][:])
                        nc.sync.dma_start(dbg_xtr, xts[0][:])

                    z_ps = psC[0:97, 64:66]
                    nc.tensor.matmul(z_ps, r(pr_quad[:]), r(ones_c[:]),
                                     start=True, stop=True,
                                     skip_group_check=True)
                    zi_sb = sm_pool.tile([97, 1], f32, tag="zi_sb")
                    nc.vector.reciprocal(zi_sb[:], z_ps[0:97, 0:1])

                    psW = psw_pool.tile([128, 512], f32, tag="psw")
                    for c in range(TC):
                        j, cl = c // 4, c % 4
                        rhs = natf[hq][j][:, 512 * cl:512 * (cl + 1)]
                        nc.tensor.matmul(psW[:], r(p_quad[:, c:c + 128]),
                                         r(rhs),
                                         start=(c == 0), stop=(c == TC - 1),
                                         skip_group_check=True)

                    if dbg and b == 0 and hq == 0:
                        nc.sync.dma_start(dbg_p, p_quad[:])

                    for q in range(4):
                        col = b * (D // 4) + hq * N
                        nc.scalar.activation(
                            out_sb[32 * q:32 * q + 1, col:col + N],
                            psW[32 * q:32 * q + 1, q * N:(q + 1) * N],
                            AF.Copy, bias=0.0,
                            scale=zi_sb[32 * q:32 * q + 1, 0:1])

            if loop_n:
                with tc.For_i(0, loop_n, 1):
                    for b in range(BL):
                        natf = load_nat(b)
                        do_batch(b, natf, downcast(natf, b))
            else:
                sched = [bb for _ in range(repeat) for bb in range(BL)]
                for it, b in enumerate(sched):
                    natf = load_nat(b)
                    do_batch(b, natf, downcast(natf, it))

            for q in range(4):
                nc.sync.dma_start(
                    out_s.rearrange("b (j q n) -> q b j n", q=4, n=N)[q:q + 1],
                    out_sb[32 * q:32 * q + 1, :].rearrange(
                        "one (b j n) -> one b j n", j=H // 4, n=N))
    return nc


def _consts(inputs):
    import ml_dtypes
    W_w = np.asarray(inputs["W_w"], dtype=np.float32)      # (K2, N)
    W_b = np.asarray(inputs["W_b"], dtype=np.float32)      # (K2,)
    Wm_w = np.asarray(inputs["Wm_w"], dtype=np.float32)    # (K2, N)
    Wm_b = np.asarray(inputs["Wm_b"], dtype=np.float32)    # (K2,)
    Wh_w = np.asarray(inputs["Wh_w"], dtype=np.float32)    # (1, K2)

    bf = ml_dtypes.bfloat16
    wgz = np.zeros((N, 32), np.float32)
    wgz[:, 0:K2] = W_w.T
    wmz = np.zeros((N, 32), np.float32)
    wmz[:, 0:K2] = Wm_w.T / T
    wbc = np.zeros((128, 1), np.float32)
    wmbc = np.zeros((128, 1), np.float32)
    whwm = np.zeros((128, 4), np.float32)
    for q in range(4):
        wbc[32 * q:32 * q + K2, 0] = W_b
        wmbc[32 * q:32 * q + K2, 0] = Wm_b
        whwm[32 * q:32 * q + K2, q] = Wh_w[0]
    return {
        "wgz": wgz.astype(bf),
        "wmz": wmz.astype(bf),
        "wb_col": wbc,
        "wmb_col": wmbc,
        "whw_mask": whwm,
        "ones_col": np.ones((128, 2), np.float32),
    }


def kernel(**inputs):
    import concourse.bass as bass
    import concourse.bacc as bacc
    import concourse.tile as tile
    import concourse.mybir as mybir
    from concourse import bass_utils

    hyp = np.ascontiguousarray(np.asarray(inputs["hyp"], dtype=np.float32))
    Wh_b = np.asarray(inputs["Wh_b"], dtype=np.float32)    # (1,)

    nc = bacc.Bacc("TRN2", target_bir_lowering=False, debug=False)
    _build(nc, tile, mybir, bass, float(Wh_b.reshape(-1)[0]))
    nc.compile()

    consts = _consts(inputs)
    in_maps = []
    for j in range(NCORES):
        m = {"hyp_s": np.ascontiguousarray(hyp[:, j * BL:(j + 1) * BL, :])}
        m.update(consts)
        in_maps.append(m)

    trace = os.environ.get("BASS_KERNEL_TRACE", "0") == "1"
    res = bass_utils.run_bass_kernel_spmd(
        nc, in_maps, core_ids=list(range(NCORES)), trace=trace)

    LAST_RESULT.clear()
    LAST_RESULT["exec_time_ns"] = res.exec_time_ns
    LAST_RESULT["trace"] = (res.instructions_and_trace[1]
                            if res.instructions_and_trace else None)
    LAST_RESULT["profile_json"] = res.profile_json

    out = np.concatenate([res.results[j]["out_s"] for j in range(NCORES)],
                         axis=0)
    return out.astype(np.float32)
